# revision 15
# baseline (speedup 1.0000x reference)
"""BitLinear (BitNet b1.58) forward kernel for Trainium2, 8 NeuronCores.

Computes  y = einsum('bsi,oi->bso', x, w_ste) + bias  where
  scale  = max(mean(|W|), 1e-8)
  w_q    = clip(round(W/scale), -1, 1)   (ternary)
  w_ste  = w_q * scale

Sharding: data-parallel over rows; each core owns one batch element
(2048 rows) and the full weight.

Quantization happens on the HOST, bit-exactly replicating the reference
(scale via jax-on-CPU mean — numpy's pairwise mean is 2 ulps off, which
flips ternary weights at the round(w/scale) boundary; with the exact
scale, numpy's round/clip reproduce the reference ternary identically).

Device: pure fp8 DoubleRow matmuls (2 contraction rows/cycle — the only
2x-rate PE mode; requires both operands fp8e4/e5). All error then comes
from e4m3-quantizing x (rel ~2.6e-2), reduced by a residual pass over
the first R of 16 k-pairs: xr16 = e4m3(16*(x - x8)) against wr =
ternary*(1/16) (both exact in e4m3; the 16x prescale keeps the residual
out of fp8-subnormal range), accumulated into the same PSUM group.
Residual coverage R trades time for error: rel ~= 2.65e-2*sqrt(1-R/16).

All DRAM operands are laid out partition-major on the host so DMA moves
large contiguous per-partition packets (4-16KB) — per-queue DMA rate is
packet-size-bound. The two hardware DGE queues (SP=nc.sync and
Activation=nc.scalar) are byte-balanced at the head (x8/xr tiles split
even/odd, w pairs alternating) and specialized afterwards (scalar: one
big weight DMA per chunk; sync: y writes). Chunk 0 runs k-major across
8 PSUM banks so the PE starts as soon as k-pair 0 lands.

Drain: y16 = psum * scale + bias fused on DVE, written fp16 (upcast to
f32 on host; fp16 rounding adds ~2e-4 rel).
"""

import numpy as np
import ml_dtypes

import concourse.tile as tile
import concourse.mybir as mybir
from concourse import bacc
from concourse.bass import ts
from concourse.bass_utils import run_bass_kernel_spmd

N_CORES = 8
IN_F = 4096
OUT_F = 4096
ROWS = 2048               # rows per core
P = 128                   # SBUF partitions
KT = IN_F // P            # 32 k-tiles
KP = KT // 2              # 16 k-pairs (DoubleRow covers 2 k-tiles)
MT = ROWS // P            # 16 row-tiles per core
OCH = 512                 # out-feature chunk = PSUM bank width
NCH = OUT_F // OCH        # 8 chunks
R = 9                     # residual k-pairs covered (of 16)

F32 = mybir.dt.float32
F16 = mybir.dt.float16
F8 = mybir.dt.float8e4
E4 = np.dtype(ml_dtypes.float8_e4m3)
DR = mybir.MatmulPerfMode.DoubleRow

LAST_RESULTS = None
_NC_CACHE = {}


def _build():
    nc = bacc.Bacc(
        "TRN2", target_bir_lowering=False, debug=False, num_devices=N_CORES
    )
    # partition-major layouts (second dim is per-partition linear bytes)
    x8 = nc.dram_tensor("x8", [P, KT * ROWS], F8, kind="ExternalInput").ap()
    w8 = nc.dram_tensor("w8", [P, NCH * KT * OCH], F8, kind="ExternalInput").ap()
    if R > 0:
        xr = nc.dram_tensor("xr", [P, 2 * R * ROWS], F8, kind="ExternalInput").ap()
        wr = nc.dram_tensor(
            "wr", [P, NCH * 2 * R * OCH], F8, kind="ExternalInput"
        ).ap()
    sc = nc.dram_tensor("sc", [1, 1], F32, kind="ExternalInput").ap()
    bias = nc.dram_tensor("bias", [1, OUT_F], F32, kind="ExternalInput").ap()
    y = nc.dram_tensor("y", [ROWS, OUT_F], F16, kind="ExternalOutput").ap()

    with tile.TileContext(nc) as tc:
        with (
            tc.tile_pool(name="xp", bufs=1) as xp,
            tc.tile_pool(name="wp", bufs=2) as wp,
            tc.tile_pool(name="bp", bufs=2) as bp,
            tc.tile_pool(name="yp", bufs=4) as yp,
            tc.tile_pool(name="psum", bufs=8, space="PSUM") as pp,
        ):
            scb = xp.tile([P, 1], F32)
            xsb = xp.tile([P, KT, ROWS], F8)
            if R > 0:
                xrb = xp.tile([P, 2 * R, ROWS], F8)

            for j in range(NCH):
                jo = j * OCH
                wt_j = wp.tile([P, KT, OCH], F8)
                if R > 0:
                    wr_j = wp.tile([P, 2 * R, OCH], F8)
                wb = j * KT * OCH
                rb = j * 2 * R * OCH
                if j == 0:
                    # head feed, consumption order, bytes balanced across the
                    # two queues: per main pair ~320KB on each (x8 even tile +
                    # half the w pairs on sync, odd tile + rest on scalar).
                    for i in range(KP):
                        nc.sync.dma_start(
                            out=xsb[:, 2 * i, :],
                            in_=x8[:, 2 * i * ROWS : (2 * i + 1) * ROWS],
                        )
                        (nc.sync if i % 2 else nc.scalar).dma_start(
                            out=wt_j[:, 2 * i : 2 * i + 2, :],
                            in_=w8[:, wb + 2 * i * OCH : wb + (2 * i + 2) * OCH],
                        )
                        nc.scalar.dma_start(
                            out=xsb[:, 2 * i + 1, :],
                            in_=x8[:, (2 * i + 1) * ROWS : (2 * i + 2) * ROWS],
                        )
                    for i in range(R):
                        nc.sync.dma_start(
                            out=xrb[:, 2 * i, :],
                            in_=xr[:, 2 * i * ROWS : (2 * i + 1) * ROWS],
                        )
                        (nc.sync if i % 2 else nc.scalar).dma_start(
                            out=wr_j[:, 2 * i : 2 * i + 2, :],
                            in_=wr[:, rb + 2 * i * OCH : rb + (2 * i + 2) * OCH],
                        )
                        nc.scalar.dma_start(
                            out=xrb[:, 2 * i + 1, :],
                            in_=xr[:, (2 * i + 1) * ROWS : (2 * i + 2) * ROWS],
                        )
                    # scale broadcast (128 tiny descriptors) rides the scalar
                    # queue after the head feed; first drain needs it ~45us in
                    nc.scalar.dma_start(
                        out=scb, in_=sc[0:1, 0:1].broadcast_to([P, 1])
                    )
                else:
                    # steady state: one big linear DMA per stream (16KB and
                    # 8KB per-partition packets), all on the scalar queue
                    nc.scalar.dma_start(out=wt_j, in_=w8[:, wb : wb + KT * OCH])
                    if R > 0:
                        nc.scalar.dma_start(
                            out=wr_j, in_=wr[:, rb : rb + 2 * R * OCH]
                        )
                bt = bp.tile([P, OCH], F32)
                nc.scalar.dma_start(
                    out=bt, in_=bias[0:1, jo : jo + OCH].broadcast_to([P, OCH])
                )

                def _drain(ps, m):
                    ysb = yp.tile([P, OCH], F16, name="ysb")
                    # fused drain: ysb = psum * scale + bias
                    nc.vector.scalar_tensor_tensor(
                        out=ysb,
                        in0=ps,
                        scalar=scb,
                        in1=bt,
                        op0=mybir.AluOpType.mult,
                        op1=mybir.AluOpType.add,
                    )
                    nc.sync.dma_start(out=y[ts(m, P), jo : jo + OCH], in_=ysb)

                if j == 0:
                    # chunk 0 overlaps the initial feed: m-tiles 0..7 go
                    # k-major across 8 PSUM banks so the PE consumes each
                    # k-pair as it lands; m-tiles 8..15 then go m-major (all
                    # data resident, and groups hand banks over one at a
                    # time instead of stalling on 8 serialized drains).
                    pss = [pp.tile([P, OCH], F32, name="ps") for mi in range(8)]
                    for i in range(KP):
                        for mi in range(8):
                            nc.tensor.matmul(
                                pss[mi],
                                xsb[:, 2 * i : 2 * i + 2, ts(mi, P)],
                                wt_j[:, 2 * i : 2 * i + 2, :],
                                start=(i == 0),
                                stop=(i == KP - 1 and R == 0),
                                perf_mode=DR,
                            )
                    for i in range(R):
                        for mi in range(8):
                            nc.tensor.matmul(
                                pss[mi],
                                xrb[:, 2 * i : 2 * i + 2, ts(mi, P)],
                                wr_j[:, 2 * i : 2 * i + 2, :],
                                start=False,
                                stop=(i == R - 1),
                                perf_mode=DR,
                            )
                    for mi in range(8):
                        _drain(pss[mi], mi)
                mrange = range(8, MT) if j == 0 else range(MT)
                if True:
                    for m in mrange:
                        ps = pp.tile([P, OCH], F32, name="ps")
                        for i in range(KP):
                            nc.tensor.matmul(
                                ps,
                                xsb[:, 2 * i : 2 * i + 2, ts(m, P)],
                                wt_j[:, 2 * i : 2 * i + 2, :],
                                start=(i == 0),
                                stop=(i == KP - 1 and R == 0),
                                perf_mode=DR,
                            )
                        for i in range(R):
                            nc.tensor.matmul(
                                ps,
                                xrb[:, 2 * i : 2 * i + 2, ts(m, P)],
                                wr_j[:, 2 * i : 2 * i + 2, :],
                                start=False,
                                stop=(i == R - 1),
                                perf_mode=DR,
                            )
                        _drain(ps, m)

    nc.compile()
    return nc


def _get_nc():
    if "nc" not in _NC_CACHE:
        _NC_CACHE["nc"] = _build()
    return _NC_CACHE["nc"]


def _ref_scale(weight):
    """max(mean(|W|), 1e-8) bit-exactly as the jax reference computes it."""
    import jax
    import jax.numpy as jnp

    with jax.default_device(jax.devices("cpu")[0]):
        s = jnp.maximum(jnp.mean(jnp.abs(weight)), 1e-8)
        return np.float32(np.asarray(s))


def _pmajor(a_t, nt):
    """[nt*P, cols] (k on rows) -> [P, nt*cols] partition-major."""
    cols = a_t.shape[1]
    return np.ascontiguousarray(
        a_t.reshape(nt, P, cols).transpose(1, 0, 2).reshape(P, nt * cols)
    )


def kernel(x, weight, bias):
    global LAST_RESULTS
    x = np.asarray(x)
    weight = np.asarray(weight, dtype=np.float32)
    bias = np.asarray(bias, dtype=np.float32)
    b, s, _ = x.shape
    rows = b * s
    assert rows == N_CORES * ROWS

    scale = _ref_scale(weight)
    # with the exact scale, numpy round/clip match the reference ternary
    tern = np.clip(np.round(weight / scale), -1.0, 1.0).astype(np.float32)
    tt = tern.T.astype(E4)                                     # [in, out] +-1
    # w8: [P, NCH*KT*OCH] — chunk-major then k-tile, linear per partition
    w8 = np.ascontiguousarray(
        tt.reshape(KT, P, NCH, OCH).transpose(1, 2, 0, 3).reshape(P, -1)
    )
    if R > 0:
        wr = np.ascontiguousarray(
            (tern.T[: 2 * R * P] * np.float32(0.0625))
            .astype(E4)
            .reshape(2 * R, P, NCH, OCH)
            .transpose(1, 2, 0, 3)
            .reshape(P, -1)
        )
    sc = np.full((1, 1), scale, dtype=np.float32)
    b2 = np.ascontiguousarray(bias.reshape(1, OUT_F))

    xf = x.reshape(rows, IN_F).astype(np.float32)
    in_maps = []
    for c in range(N_CORES):
        xs = xf[c * ROWS : (c + 1) * ROWS]
        x8c = xs.astype(E4)
        m = {
            "x8": _pmajor(np.ascontiguousarray(x8c.T), KT),
            "w8": w8,
            "sc": sc,
            "bias": b2,
        }
        if R > 0:
            xr16 = ((xs - x8c.astype(np.float32)) * np.float32(16.0)).astype(E4)
            m["xr"] = _pmajor(np.ascontiguousarray(xr16.T[: 2 * R * P]), 2 * R)
            m["wr"] = wr
        in_maps.append(m)

    nc = _get_nc()
    try:
        res = run_bass_kernel_spmd(nc, in_maps, core_ids=list(range(N_CORES)))
    except Exception:
        # transient device wedge (NRT_EXEC_UNIT_UNRECOVERABLE) — one retry
        import time

        time.sleep(5.0)
        res = run_bass_kernel_spmd(nc, in_maps, core_ids=list(range(N_CORES)))
    LAST_RESULTS = res
    y = np.concatenate([res.results[c]["y"] for c in range(N_CORES)], axis=0)
    return np.ascontiguousarray(y.reshape(b, s, OUT_F).astype(np.float32))
